# revision 44
# baseline (speedup 1.0000x reference)
"""CapsNet (semantic capsules + dynamic routing) on 8 TRN2 NeuronCores.

Sharding: sequence-shard the fc1/squash stage (each core owns 32 of 256
sequence positions = 256 of 2048 contraction elements), AllGather the
transposed u tensor in 2-route chunks (3 chunks for R=6) so stage B
(priors) for the first chunk starts while later chunks are still on
the wire -- each chunk's ~14-17us wire time matches one B-chunk's
~18us of matmuls, so the collective stream and the PE pipeline in
lockstep. Core i computes capsule i's priors with full K locally, runs
dynamic routing for capsule i, and emits output batches 8i..8i+8 (the
reference's flat reinterpret of vote maps capsule i onto exactly those
batches).

Stage A pipelining: fc1 runs j-outer (contraction tile outer) so the
matmuls chase the xt DMAs; the sem->u transposes use s-pair [80,128]
blocks (16 instead of 32 transposes) producing a (parity, t, c) local-k
order that the host-side route_weights permutation compensates, so the
uT chunks feed the AllGather ~25us earlier than the h-split layout.

DMA scheduling (all bulk traffic rides the single in-order SP queue so
relative priority is explicit): xt tiles first (fc1-pacing), then
route_weights kt 0..11 (host pre-transposed to partition-major so the
transfer is one contiguous descriptor; a strided layout costs 30us+ of
descriptor generation), then the queue PARKS on the ug_in data
semaphore -- the tiny AllGather payload hits the wire the moment stage
A produces it, and the rw kt 12..15 tail streams during the
collective's multi-core rendezvous delay (stage B only reads those
columns ~20us into its matmul loop). ug_big loads are per-kt so stage
B group 0 starts on the first gathered tile. Engine-queue DMAs other
than SP are avoided for latency-critical transfers: gpsimd/scalar DGE
queues measured 10-25x slower, and dependency-free DMAs on a side
queue get converted to static descriptors that fire at kernel start,
stealing xt's HBM bandwidth.

Precision: the routing softmax has near-tie logits at scale ~200, so
priors need ~16+ mantissa bits -- plain bf16/fp16/fp32r all flip the
ties and blow the 2e-2 gate (f32r measured 0.045). Stage B therefore
uses a split-bf16 scheme: u = uh + ul and rw = rwh + rwl (each half
bf16, residual computed against the rounded high half), keeping the
three significant cross terms uh@rwh + ul@rwh + uh@rwl. That carries
~16 mantissa bits while the PE streams bf16 rows 4x faster than fp32,
cutting stage B from ~53us to ~40us. fc1 uses the same 3-term split
(x = xh+xl, fc1_w split host-side for free), cutting its serial PE
time 35us -> 26us. Combined rel err measured 0.0122 (deterministic
for the fixed-seed input) vs 0.0045 full-fp32 and the 2e-2 gate. The
squash/routing chain stays fp32; the final vote @ larger_w matmul and
the output are bf16.
"""
import sys
from contextlib import ExitStack

if '/opt/trn_rl_repo' not in sys.path:
    sys.path.insert(0, '/opt/trn_rl_repo')

import numpy as np
import ml_dtypes

import concourse.bass as bass
import concourse.bacc as bacc
import concourse.tile as tile
from concourse import mybir
import concourse.bass_utils as bass_utils

F32 = mybir.dt.float32
F32R = mybir.dt.float32r
BF16 = mybir.dt.bfloat16
AX = mybir.AxisListType
ALU = mybir.AluOpType
ACTF = mybir.ActivationFunctionType

N_CORES = 8
B, S, D = 64, 256, 768
CAP, NT = 8, 10
NCOL = NT * CAP          # 80 fc1 output cols (n*8+c)
SL = S // N_CORES        # 32 sequence positions per core
KL = SL * CAP            # 256 local contraction elements
KT = 16                  # global k tiles of 128 (K = S*CAP = 2048)
TP = SL // 2             # 16 s-pair tiles per core
L = S                    # 256 class dim
BLOC = B // N_CORES      # 8 output batches per core

_cache = {}


def _build(R: int, debug_mode=0):
    """Build + compile the SPMD program for R active routes.

    debug_mode: 0 normal; 3 dump priors; 4 dump vote.
    """
    # 2 AG chunks of 3 routes: with split-bf16 stage B the pipeline is
    # AG-wire-gated, and bigger chunks move bytes at better link
    # efficiency (~85 vs ~70 GB/s) than 2-route chunks
    RA = (R + 1) // 2
    RB = R - RA
    NGRP = 2 if RB > 0 else 1
    RG_N = [RA, RB][:NGRP]
    RG_BASE = [0, RA][:NGRP]

    nc = bacc.Bacc("TRN2", target_bir_lowering=False, debug=False,
                   num_devices=N_CORES)

    xt = nc.dram_tensor("xt", [D, 2 * SL * B], BF16,
                        kind="ExternalInput")
    fw = nc.dram_tensor("fw", [128, 2 * 6 * NCOL], BF16,
                        kind="ExternalInput")
    fb = nc.dram_tensor("fb", [NCOL, 1], F32, kind="ExternalInput")
    rwh = nc.dram_tensor("rwh", [128, KT * R * L], BF16,
                         kind="ExternalInput")
    rwl = nc.dram_tensor("rwl", [128, KT * R * L], BF16,
                         kind="ExternalInput")
    lwtp = nc.dram_tensor("lwtp", [4, 32, D], BF16, kind="ExternalInput")
    ident = nc.dram_tensor("ident", [128, 128], F32, kind="ExternalInput")
    out = nc.dram_tensor("out", [BLOC * S, D], BF16, kind="ExternalOutput")

    dum_in = nc.dram_tensor("dum_in", [128, 2], BF16)
    dum_out = nc.dram_tensor("dum_out", [N_CORES, 128, 2], BF16,
                             addr_space="Shared")
    ug_in = [nc.dram_tensor(f"ug_in{g}", [128, 4 * RG_N[g] * B], BF16)
             for g in range(NGRP)]
    ug_out = [nc.dram_tensor(f"ug_out{g}", [N_CORES, 128, 4 * RG_N[g] * B],
                             BF16, addr_space="Shared")
              for g in range(NGRP)]

    ecnt = [0]

    def copy_rr(dst, src, engines=('v', 'a')):
        """Round-robin PSUM->SBUF copies across engines."""
        ecnt[0] += 1
        e = engines[ecnt[0] % len(engines)]
        if e == 'v':
            nc.vector.tensor_copy(dst, src)
        elif e == 'a':
            nc.scalar.copy(dst, src)
        else:
            nc.gpsimd.tensor_copy(dst, src)

    with tile.TileContext(nc) as tc:
        with (
            tc.tile_pool(name="const", bufs=1) as constp,
            tc.tile_pool(name="junk", bufs=6) as junkp,
            tc.tile_pool(name="rwp", bufs=1) as rwp,
            tc.tile_pool(name="route", bufs=1) as rt,
            tc.tile_pool(name="acc", bufs=2) as accp,
        ):
            sa_stack = ExitStack()
            xtp = sa_stack.enter_context(tc.tile_pool(name="xtp", bufs=1))
            sa = sa_stack.enter_context(tc.tile_pool(name="stageA", bufs=1))

            # warm the CC cores: a 512B dummy AllGather fired at kernel
            # start absorbs the first-collective wakeup/setup delay
            # (~10us) in fc1's shadow; the real chunk 0 queues behind it
            # on the stream with the CC path already hot
            nc.gpsimd.collective_compute(
                "AllGather", ALU.bypass,
                replica_groups=[list(range(N_CORES))],
                ins=[dum_in[:]], outs=[dum_out[:]],
            )

            # ---- bulk input DMA on the SP queue in need-order: consts,
            # xt (6 tiles so fc1 can chase arrivals), then rw as one
            # batched transfer (needed only by stage B, ~60us later).
            xt_t = []
            fw_sb = constp.tile([128, 2 * 6 * NCOL], BF16, tag="fw")
            for j in range(6):
                t = xtp.tile([128, 2 * SL * B], BF16, tag=f"xt{j}",
                             name=f"xt{j}")
                nc.sync.dma_start(out=t[:], in_=xt[j * 128:(j + 1) * 128, :])
                xt_t.append(t)
                if j == 0:
                    nc.sync.dma_start(out=fw_sb[:], in_=fw[:])
            fb_sb = constp.tile([NCOL, 1], F32, tag="fb")
            nc.sync.dma_start(out=fb_sb[:], in_=fb[:])
            id_sb = constp.tile([128, 128], F32, tag="ident")
            nc.sync.dma_start(out=id_sb[:], in_=ident[:])
            RWH = 10 * R * L          # rw kt 0..9 now; kt 10..15 are
            rwh_sb = rwp.tile([128, KT * R * L], BF16, tag="rwh")
            rwl_sb = rwp.tile([128, KT * R * L], BF16, tag="rwl")
            nc.sync.dma_start(out=rwh_sb[:, 0:RWH], in_=rwh[:, 0:RWH])
            nc.sync.dma_start(out=rwl_sb[:, 0:RWH], in_=rwl[:, 0:RWH])
            lwtp_sb = []
            for m in range(4):
                t = constp.tile([32, D], BF16, tag=f"lwtp{m}", name=f"lwtp{m}")
                nc.sync.dma_start(out=t[:], in_=lwtp[m])
                lwtp_sb.append(t)

            # ================= stage A: fc1 -> uT =====================
            ps_stack = ExitStack()
            psA = ps_stack.enter_context(
                tc.tile_pool(name="psA", bufs=1, space="PSUM"))
            psT = ps_stack.enter_context(
                tc.tile_pool(name="psT", bufs=3, space="PSUM"))

            # fc1: j (contraction) outer so compute overlaps the xt
            # DMAs; split-bf16 (x = xh+xl, fw = fwh+fwl host-side, keep
            # xh@fwh + xl@fwh + xh@fwl) streams 3 bf16 passes in 3/4 of
            # the fp32 cycles at the same ~16-bit precision
            psum_sem = psA.tile([NCOL, SL * B], F32, tag="sem")
            for j in range(6):
                for tm in range(3):
                    xhl = 1 if tm == 1 else 0
                    fhl = 1 if tm == 2 else 0
                    for n4 in range(4):
                        nc.tensor.matmul(
                            psum_sem[:, n4 * 512:(n4 + 1) * 512],
                            lhsT=fw_sb[:, fhl * 6 * NCOL + j * NCOL:
                                       fhl * 6 * NCOL + (j + 1) * NCOL],
                            rhs=xt_t[j][:, xhl * SL * B + n4 * 512:
                                        xhl * SL * B + (n4 + 1) * 512],
                            start=(j == 0 and tm == 0),
                            stop=(j == 5 and tm == 2),
                        )

            semT_sb = sa.tile([NCOL, SL * B], F32, tag="semT")
            for n4 in range(4):
                sl_ = slice(n4 * 512, (n4 + 1) * 512)
                nc.vector.tensor_scalar_add(
                    semT_sb[:, sl_], psum_sem[:, sl_], fb_sb[0:NCOL, 0:1])

            # s-pair transposes: [80, 128] -> [128 = (s2 b), 80 = nc]
            u_all = sa.tile([128, TP * NCOL], F32, tag="u_all")
            for t in range(TP):
                ps_t = psT.tile([128, NCOL], F32, tag="pst", bufs=2)
                nc.tensor.transpose(
                    ps_t[:], semT_sb[:, t * 128:(t + 1) * 128],
                    id_sb[0:NCOL, 0:NCOL])
                copy_rr(u_all[:, t * NCOL:(t + 1) * NCOL], ps_t[:])

            # squash over n at full 128-partition utilization
            tsq = sa.tile([128, TP * NCOL], F32, tag="tsq")
            sq = sa.tile([128, TP * CAP], F32, tag="sq")
            scl = sa.tile([128, TP * CAP], F32, tag="scl")
            TH = TP * NCOL // 2
            CH = TP * CAP // 2
            for hh in range(2):
                nc.gpsimd.tensor_mul(tsq[:, hh * TH:(hh + 1) * TH],
                                     u_all[:, hh * TH:(hh + 1) * TH],
                                     u_all[:, hh * TH:(hh + 1) * TH])
                nc.vector.tensor_reduce(
                    out=sq[:, hh * CH:(hh + 1) * CH].rearrange(
                        "p (t c) -> p t c", c=CAP),
                    in_=tsq[:, hh * TH:(hh + 1) * TH].rearrange(
                        "p (t n c) -> p t c n", n=NT, c=CAP),
                    axis=AX.X, op=ALU.add,
                )
            # scl chain split by t-halves so half 0's sqrt/recip work
            # overlaps half 1's squash reduce (same math, disjoint cols)
            rscr = sa.tile([128, TP * CAP], F32, tag="rscr")
            for hh in range(2):
                hs = slice(hh * CH, (hh + 1) * CH)
                nc.scalar.activation(scl[:, hs], sq[:, hs], ACTF.Sqrt)
                nc.vector.tensor_scalar_add(sq[:, hs], sq[:, hs], 1.0)
                nc.vector.reciprocal_approx_accurate(sq[:, hs], sq[:, hs],
                                                     rscr[:, hs])
                nc.vector.tensor_mul(scl[:, hs], scl[:, hs], sq[:, hs])

            # u_act + uT transposes, grouped by route chunk so chunk 0's
            # AllGather flies while chunk 1 is still transposing.
            # u_act[:, r*128 + t*8+c] = u[(s2 b), r, t, c] * scl
            # transpose -> [128 = (t c), 128 = (s2 b)] = local k tile
            # pair (parity s2), staged into ug_sb[(s2, rl, b)].
            u_act = sa.tile([128, R * TP * CAP], F32, tag="u_act")
            uview = u_all[:].rearrange("p (t n c) -> p n t c", n=NT, c=CAP)
            ug_sb = []
            for g in range(NGRP):
                rg = RG_N[g]
                # ugt cols = (s2, hl, rl, b): per global kt = (m, s2) the
                # gathered s2-half holds (hl, rl, b) for stage B's
                # split-bf16 triple matmuls
                ugt = sa.tile([128, 4 * rg * B], BF16, tag=f"ug{g}",
                              name=f"ug{g}")
                ug_sb.append(ugt)
                for rl in range(rg):
                    r = RG_BASE[g] + rl
                    rs = slice(r * 128, (r + 1) * 128)
                    eng = nc.vector if rl % 2 == 0 else nc.gpsimd
                    eng.tensor_mul(u_act[:, rs], uview[:, r], scl[:])
                    psU = psT.tile([128, 128], F32, tag="pst", bufs=2)
                    nc.tensor.transpose(psU[:], u_act[:, rs], id_sb[:])
                    # split on evac: uh = bf16(u) (scalar cast-copy),
                    # ul = bf16(u - uh) (vector sub) -- ~16 mantissa
                    # bits together; the PE streams bf16 at 4x fp32
                    for s2 in range(2):
                        h_dst = ugt[:, (s2 * 2 + 0) * rg * B + rl * B:
                                    (s2 * 2 + 0) * rg * B + (rl + 1) * B]
                        l_dst = ugt[:, (s2 * 2 + 1) * rg * B + rl * B:
                                    (s2 * 2 + 1) * rg * B + (rl + 1) * B]
                        nc.scalar.copy(h_dst, psU[:, s2 * B:(s2 + 1) * B])
                        nc.vector.tensor_sub(
                            l_dst, psU[:, s2 * B:(s2 + 1) * B], h_dst)
                nc.sync.dma_start(out=ug_in[g][:], in_=ugt[:])
                nc.gpsimd.collective_compute(
                    "AllGather", ALU.bypass,
                    replica_groups=[list(range(N_CORES))],
                    ins=[ug_in[g][:]], outs=[ug_out[g][:]],
                )
            # rw tail issued on the SAME sync queue right behind the
            # ug_in DMAs: the queue parks on ug_in's data semaphore, so
            # this streams during the collective rendezvous delay and is
            # done before the AllGather's wire time (B0 reads kt12-15
            # only ~20us into its matmul loop).
            nc.sync.dma_start(out=rwh_sb[:, RWH:], in_=rwh[:, RWH:])
            nc.sync.dma_start(out=rwl_sb[:, RWH:], in_=rwl[:, RWH:])
            ps_stack.close()

            # ====== stage B: full-K priors, chunk-pipelined ===========
            pri = rt.tile([B, R * L], F32, tag="pri")
            ssum = rt.tile([B, L], F32, tag="ssum")

            def pri_rr(r):
                return pri[:, r * L:(r + 1) * L]
            sa_stack.close()
            ugp_stack = ExitStack()
            ugp = ugp_stack.enter_context(tc.tile_pool(name="ugp", bufs=1))
            psb_stack = ExitStack()
            for g in range(NGRP):
                rg = RG_N[g]
                ug_big = ugp.tile([128, KT * 2 * rg * B], BF16,
                                  tag=f"ugb{g}", name=f"ugb{g}")
                for kt in range(KT):
                    nc.sync.dma_start(
                        out=ug_big[:, kt * 2 * rg * B:
                                   (kt + 1) * 2 * rg * B],
                        in_=ug_out[g][kt // 2, :,
                                      (kt % 2) * 2 * rg * B:
                                      (kt % 2 + 1) * 2 * rg * B])
                psB = psb_stack.enter_context(
                    tc.tile_pool(name=f"psB{g}", bufs=1, space="PSUM"))
                psb_t = [psB.tile([B, L], F32, tag=f"pb{rl}", name=f"pb{rl}")
                         for rl in range(rg)]
                for kt in range(KT):
                    for tm in range(3):
                        # (uh,rwh), (ul,rwh), (uh,rwl); ul@rwl ~2^-16 drop
                        hl = 1 if tm == 1 else 0
                        rw_t = rwl_sb if tm == 2 else rwh_sb
                        for rl in range(rg):
                            nc.tensor.matmul(
                                psb_t[rl][:],
                                lhsT=ug_big[:, kt * 2 * rg * B
                                            + hl * rg * B + rl * B:
                                            kt * 2 * rg * B
                                            + hl * rg * B + (rl + 1) * B],
                                rhs=rw_t[:, (kt * R + RG_BASE[g] + rl) * L:
                                         (kt * R + RG_BASE[g] + rl + 1)
                                         * L],
                                start=(kt == 0 and tm == 0),
                                stop=(kt == KT - 1 and tm == 2),
                            )
                for rl in range(rg):
                    r = RG_BASE[g] + rl
                    copy_rr(pri[:, r * L:(r + 1) * L], psb_t[rl][:])
                # route-sum: group 0's partial accumulates on gpsimd
                # under group 1's matmuls; group 1's tail is pairwise
                # across gpsimd+vector so only ~2 add-depths are
                # exposed after the last priors copy
                b0 = RG_BASE[g]
                if g == 0:
                    nc.gpsimd.tensor_copy(ssum[:], pri_rr(b0))
                    for rl in range(1, rg):
                        nc.gpsimd.tensor_add(ssum[:], ssum[:],
                                             pri_rr(b0 + rl))
                elif rg == 3:
                    t1 = rt.tile([B, L], F32, tag="sst1", name="sst1")
                    nc.gpsimd.tensor_add(t1[:], pri_rr(b0), pri_rr(b0 + 1))
                    t2 = rt.tile([B, L], F32, tag="sst2", name="sst2")
                    nc.vector.tensor_add(t2[:], ssum[:], pri_rr(b0 + 2))
                    nc.vector.tensor_add(ssum[:], t1[:], t2[:])
                else:
                    for rl in range(rg):
                        nc.gpsimd.tensor_add(ssum[:], ssum[:],
                                             pri_rr(b0 + rl))
            psb_stack.close()
            ugp_stack.close()

            if debug_mode == 3:
                nc.gpsimd.dma_start(out=out[0:B, 0:D], in_=pri[:, 0:D])
                nc.gpsimd.dma_start(out=out[B:2 * B, 0:R * L - D],
                                    in_=pri[:, D:R * L])

            if debug_mode in (0, 4):
                # ============= stage C: dynamic routing ================
                def pri_r(r):
                    return pri[:, r * L:(r + 1) * L]

                # iter 0: probs uniform; ssum pre-accumulated in stage B
                logits = rt.tile([B, R], F32, tag="logits")
                vote = rt.tile([B, L], F32, tag="vote")

                def squash_scale(v, sqscale, tag, extra_scale=1.0):
                    """[B,1] tile: extra_scale*sqrt(sq)/(1+sq), with
                    sq = sum(v*v)*sqscale."""
                    junk = junkp.tile([B, L], F32, tag="junk", name="junk")
                    sqr = rt.tile([B, 1], F32, tag=tag + "sr", name=tag + "sr")
                    nc.vector.scalar_tensor_tensor(
                        out=junk[:], in0=v, scalar=1.0, in1=v,
                        op0=ALU.mult, op1=ALU.mult, accum_out=sqr[:])
                    if sqscale != 1.0:
                        sqv = rt.tile([B, 1], F32, tag=tag + "sq",
                                      name=tag + "sq")
                        nc.vector.tensor_scalar_mul(sqv[:], sqr[:],
                                                    float(sqscale))
                    else:
                        sqv = sqr
                    a = rt.tile([B, 1], F32, tag=tag + "a", name=tag + "a")
                    nc.scalar.activation(a[:], sqv[:], ACTF.Sqrt)
                    bb = rt.tile([B, 1], F32, tag=tag + "b", name=tag + "b")
                    nc.vector.tensor_scalar_add(bb[:], sqv[:], 1.0)
                    cc = rt.tile([B, 1], F32, tag=tag + "c", name=tag + "c")
                    nc.vector.reciprocal(cc[:], bb[:])
                    sc = rt.tile([B, 1], F32, tag=tag + "s", name=tag + "s")
                    if extra_scale != 1.0:
                        nc.vector.scalar_tensor_tensor(
                            out=sc[:], in0=a[:], scalar=float(extra_scale),
                            in1=cc[:], op0=ALU.mult, op1=ALU.mult)
                    else:
                        nc.vector.tensor_mul(sc[:], a[:], cc[:])
                    return sc

                def raw_delta(vsrc, dst):
                    """dst[b, r] = sum_l pri_r * vsrc."""
                    for r in range(R):
                        junk = junkp.tile([B, L], F32, tag="junk", name="junk")
                        nc.vector.scalar_tensor_tensor(
                            out=junk[:], in0=pri_r(r), scalar=1.0, in1=vsrc,
                            op0=ALU.mult, op1=ALU.mult,
                            accum_out=dst[:, r:r + 1])

                def softmax(lg, tag):
                    ngm = rt.tile([B, 1], F32, tag=tag + "ng", name=tag + "ng")
                    nc.vector.tensor_reduce(out=ngm[:], in_=lg[:], axis=AX.X,
                                            op=ALU.max, negate=True)
                    ex = rt.tile([B, R], F32, tag=tag + "ex", name=tag + "ex")
                    nc.scalar.activation(ex[:], lg[:], ACTF.Exp,
                                         bias=ngm[0:B, 0:1])
                    se = rt.tile([B, 1], F32, tag=tag + "se", name=tag + "se")
                    nc.vector.tensor_reduce(out=se[:], in_=ex[:], axis=AX.X,
                                            op=ALU.add)
                    ri = rt.tile([B, 1], F32, tag=tag + "ri", name=tag + "ri")
                    nc.vector.reciprocal(ri[:], se[:])
                    pr = rt.tile([B, R], F32, tag=tag + "pr", name=tag + "pr")
                    nc.vector.tensor_scalar_mul(pr[:], ex[:], ri[0:B, 0:1])
                    return pr

                def vote_chain(pr, vdst, lsl):
                    """vdst = sum_r probs_r * pri_r[:, lsl], as a tree:
                    leaf products via tensor_scalar_mul, pair-merges via
                    fused STT, so the serial depth is ~3 not R."""
                    n = lsl.stop - lsl.start
                    terms = []
                    for r0 in range(0, R, 2):
                        acc = accp.tile([B, L], F32, tag="acc", name="acc",
                                        bufs=4)
                        nc.vector.tensor_scalar_mul(
                            acc[:, 0:n], pri_r(r0)[:, lsl],
                            pr[0:B, r0:r0 + 1])
                        if r0 + 1 < R:
                            acc2 = accp.tile([B, L], F32, tag="acc",
                                             name="acc", bufs=4)
                            nc.vector.scalar_tensor_tensor(
                                out=acc2[:, 0:n], in0=pri_r(r0 + 1)[:, lsl],
                                scalar=pr[0:B, r0 + 1:r0 + 2],
                                in1=acc[:, 0:n], op0=ALU.mult, op1=ALU.add)
                            acc = acc2
                        terms.append(acc)
                    while len(terms) > 1:
                        nxt = []
                        for i in range(0, len(terms) - 1, 2):
                            s = accp.tile([B, L], F32, tag="acc", name="acc",
                                          bufs=4)
                            nc.vector.tensor_add(s[:, 0:n],
                                                 terms[i][:, 0:n],
                                                 terms[i + 1][:, 0:n])
                            nxt.append(s)
                        if len(terms) % 2 == 1:
                            nxt.append(terms[-1])
                        terms = nxt
                    nc.vector.tensor_copy(vdst, terms[0][:, 0:n])

                # iter 0 (1/R for the uniform-probs vote folded into sc0)
                sc0 = squash_scale(ssum[:], 1.0 / (R * R), "i0",
                                   extra_scale=1.0 / R)
                rd0 = rt.tile([B, R], F32, tag="rd0")
                raw_delta(ssum[:], rd0)
                nc.vector.tensor_scalar_mul(logits[:], rd0[:], sc0[0:B, 0:1])

                # iter 1
                pr1 = softmax(logits, "s1")
                vote_chain(pr1, vote[:], slice(0, L))
                sc1 = squash_scale(vote[:], 1.0, "i1")
                rd1 = rt.tile([B, R], F32, tag="rd1")
                raw_delta(vote[:], rd1)
                t1 = rt.tile([B, R], F32, tag="t1")
                nc.vector.tensor_scalar_mul(t1[:], rd1[:], sc1[0:B, 0:1])
                lg2 = rt.tile([B, R], F32, tag="lg2")
                nc.vector.tensor_add(lg2[:], logits[:], t1[:])

                # iter 2: final softmax; vote computed in two l-halves so
                # stage D starts on the first half while the second runs
                pr2 = softmax(lg2, "s2")

            if debug_mode == 4:
                vote_chain(pr2, vote[:], slice(0, L))
                nc.gpsimd.dma_start(out=out[0:B, 0:L], in_=vote[:])

            if debug_mode == 0:
                # ============= stage D: reinterpret + final matmul =====
                # pack sl and sl+8 into one M=128 matmul (halves the PE
                # row-streaming and the PSUM-evac copies; both use the
                # same padded-lwt rhs), stage each half in SBUF, and emit
                # one DMA of 12KB-contiguous runs per half
                ps_stack = ExitStack()
                psT2 = ps_stack.enter_context(
                    tc.tile_pool(name="psT2", bufs=2, space="PSUM"))
                psO = ps_stack.enter_context(
                    tc.tile_pool(name="psO", bufs=3, space="PSUM"))
                oall = ps_stack.enter_context(tc.tile_pool(name="oall",
                                                           bufs=1))
                # out row = jp*32 + lh*16 + s8*8 + k
                outj = out[:].rearrange("(jp sl) d -> jp sl d", sl=SL)
                for lh in range(2):
                    lsl = slice(lh * 128, (lh + 1) * 128)
                    vote_chain(pr2, vote[:, lsl], lsl)
                    o_half = oall.tile([128, 8 * D], BF16, tag=f"oh{lh}",
                                       name=f"oh{lh}")
                    for qq in range(2):
                        # vT2 = vote quadrants q (rows 0:64) and q+2 (64:128)
                        vT2 = rt.tile([32, 2 * B], BF16, tag=f"voteQ{qq}",
                                      name=f"voteQ{lh}_{qq}")
                        for s8 in range(2):
                            q = 4 * lh + qq + 2 * s8
                            psV = psT2.tile([32, B], F32, tag="psv")
                            nc.tensor.transpose(
                                psV[:], vote[:, q * 32:(q + 1) * 32],
                                id_sb[0:B, 0:B])
                            copy_rr(vT2[:, s8 * B:(s8 + 1) * B], psV[:])
                        for m in range(4):
                            k = qq * 4 + m
                            pso = psO.tile([2 * B, D], F32, tag="pso")
                            nc.tensor.matmul(
                                pso[:, 0:512], lhsT=vT2[:],
                                rhs=lwtp_sb[m][:, 0:512],
                                start=True, stop=True)
                            nc.tensor.matmul(
                                pso[:, 512:D], lhsT=vT2[:],
                                rhs=lwtp_sb[m][:, 512:D],
                                start=True, stop=True)
                            copy_rr(o_half[:, k * D:(k + 1) * D], pso[:])
                        # out rows 16lh+8s8+4qq..+4 are exactly this
                        # quarter's k columns: DMA them now so the final
                        # quarter's transfer is the only tail left
                        for s8 in range(2):
                            base = 16 * lh + 8 * s8 + 4 * qq
                            nc.sync.dma_start(
                                out=outj[:, base:base + 4, :],
                                in_=o_half[s8 * B:(s8 + 1) * B,
                                           qq * 4 * D:(qq * 4 + 4) * D])
                ps_stack.close()

    nc.compile()
    return nc


def _prep_inputs(x, task, fc1_w, fc1_b, route_weights, larger_w):
    R = int(task) + 1
    bf = ml_dtypes.bfloat16
    fw32 = np.ascontiguousarray(
        fc1_w.reshape(NCOL, D).T.reshape(6, 128, NCOL).transpose(1, 0, 2)
    ).reshape(128, 6 * NCOL).astype(np.float32)
    fwh = fw32.astype(bf)
    fwl = (fw32 - fwh.astype(np.float32)).astype(bf)
    fw = np.concatenate([fwh, fwl], axis=1)
    fb = np.ascontiguousarray(fc1_b.reshape(NCOL, 1)).astype(np.float32)
    lwt = np.ascontiguousarray(larger_w.T).astype(np.float32)
    lwtp = np.zeros((4, 32, D), dtype=bf)
    for m in range(4):
        lwtp[m, 8 * m:8 * m + CAP] = lwt
    ident = np.eye(128, dtype=np.float32)
    in_maps = []
    for i in range(N_CORES):
        xt32 = np.ascontiguousarray(
            x[:, i * SL:(i + 1) * SL, :].transpose(2, 1, 0)
        ).reshape(D, SL * B).astype(np.float32)
        xth = xt32.astype(bf)
        xtl = (xt32 - xth.astype(np.float32)).astype(bf)
        xt_i = np.concatenate([xth, xtl], axis=1)
        # partition-major [p=(t,c), kt=(m,s2), r, l]: one contiguous
        # DMA; kt = 2m + s2 where s_global = 32m + 2t + s2. Split into
        # bf16 high/low halves (rw = rwh + rwl to ~16 mantissa bits)
        # for the 4x-faster bf16 PE streaming in stage B.
        rw_i = np.ascontiguousarray(
            route_weights[i, :R].reshape(R, 8, TP, 2, CAP, L)
            .transpose(2, 4, 1, 3, 0, 5)
        ).reshape(128, KT * R * L).astype(np.float32)
        rwh_i = rw_i.astype(bf)
        rwl_i = (rw_i - rwh_i.astype(np.float32)).astype(bf)
        in_maps.append({"xt": xt_i, "fw": fw, "fb": fb, "rwh": rwh_i,
                        "rwl": rwl_i, "lwtp": lwtp, "ident": ident})
    return in_maps


def kernel(x, task, fc1_w, fc1_b, route_weights, larger_w, larger_b,
           _return_results=False):
    x = np.asarray(x, dtype=np.float32)
    fc1_w = np.asarray(fc1_w, dtype=np.float32)
    fc1_b = np.asarray(fc1_b, dtype=np.float32)
    route_weights = np.asarray(route_weights, dtype=np.float32)
    larger_w = np.asarray(larger_w, dtype=np.float32)
    larger_b = np.asarray(larger_b, dtype=np.float32)
    R = int(task) + 1

    if R not in _cache:
        _cache[R] = _build(R)
    nc = _cache[R]

    in_maps = _prep_inputs(x, task, fc1_w, fc1_b, route_weights, larger_w)
    res = bass_utils.run_bass_kernel_spmd(nc, in_maps, list(range(N_CORES)))

    full = np.empty((B, S, D), dtype=np.float32)
    for i in range(N_CORES):
        full[i * BLOC:(i + 1) * BLOC] = np.asarray(
            res.results[i]["out"], dtype=np.float32).reshape(BLOC, S, D)
    if np.any(larger_b):
        full = full + larger_b[None, None, :]
    if _return_results:
        return full, res
    return full
